# revision 2
# baseline (speedup 1.0000x reference)
"""AttnBlock (GroupNorm + single-head spatial self-attention + residual) on
8 Trainium2 NeuronCores.

Sharding: batch (4) x query-half (2) -> 8 independent shards, one per core.
Every core runs the SAME program on different data: the host rolls the
flattened spatial axis by 2048 for odd cores so each core's queries are the
first 2048 columns of its local x, while K/V see the full 4096.

The host precomputes all per-channel affine constants (GroupNorm mean/rstd
folded with gn affine into the conv weights, conv biases folded with the GN
shift, proj+v-bias folded into the residual) so the device does only the
dense work: Q/K/V 1x1 convs, scores, softmax, PV, and the residual add.

Device pipeline (per core), everything fp8e4 DoubleRow on the PE at
256-wide contraction (0.5 cycles/row):
  1. Q/K convs: DR matmuls vs pre-scaled fp8 weights; PSUM->SBUF cast with
     the folded bias on DVE emits q/k directly in fp8 [c_lo, c_hi, n].
  2. V conv emitted transposed [n, c] (lhsT = x tile) -> vT fp8, which is
     also pre-multiplied by the proj weight (wpv = wv^T wp^T), so the PV
     matmul directly produces the proj output.
  3. Attention per 512-query chunk: ST[j,i] = k^T q (DR), one ACT exp per
     256-key pair reads [128,1024] of PSUM and writes fp8 P directly
     (exp(s/16 - 3); the -3 keeps max P ~ 96 << 240 = fp8e4 max, softmax
     shift-invariance cancels it), PV accumulates [c,i] over key pairs.
     Softmax denominator Z via a tiny all-ones [128,2,1] DR stationary
     (out row [1,512]), reciprocal on that row + GpSimd partition
     broadcast, deferred one chunk so the cross-engine latency hides
     under the next chunk's matmul/exp stream.
  4. o = a * (1/Z) + (x + bias) (DVE), DMA out per chunk.
"""
import numpy as np
import ml_dtypes

B, C, H, W = 4, 256, 64, 64
N = H * W            # 4096 spatial positions
NQ = N // 2          # 2048 queries per core
P = 128              # partitions
CT = C // P          # 2 channel tiles
NUM_GROUPS = 8
EPS = 1e-5
SCALE = float(C) ** -0.5
EBIAS = -3.0         # exp(s/16 - 3): max scaled score ~7.6 -> max P ~ 96
IC_W = 512
NIC = NQ // IC_W     # 4 query chunks
NJP = N // 256       # 16 key pairs

F8 = ml_dtypes.float8_e4m3
BF16 = ml_dtypes.bfloat16

_CACHED = {}


def _build():
    import concourse.bass as bass
    import concourse.mybir as mybir
    import concourse.tile as tile
    from concourse import bacc

    dt = mybir.dt
    AF = mybir.ActivationFunctionType
    DR = mybir.MatmulPerfMode.DoubleRow

    nc = bacc.Bacc("TRN2", debug=False, num_devices=8)

    xf8_d = nc.dram_tensor("xf8", [P, CT * N], dt.float8e4, kind="ExternalInput")
    wq_d = nc.dram_tensor("wscq", [P, CT * C], dt.float8e4, kind="ExternalInput")
    wk_d = nc.dram_tensor("wsck", [P, CT * C], dt.float8e4, kind="ExternalInput")
    wv_d = nc.dram_tensor("wscv", [P, CT * C], dt.float8e4, kind="ExternalInput")
    aux_d = nc.dram_tensor("aux", [P, 8], dt.float32, kind="ExternalInput")
    xb_d = nc.dram_tensor("xb", [P, CT * NQ], dt.bfloat16, kind="ExternalInput")
    out_d = nc.dram_tensor("out", [C, NQ], dt.float32, kind="ExternalOutput")

    out_ap = out_d.ap().rearrange("(t p) n -> p t n", p=P)

    with tile.TileContext(nc) as tc:
        with (
            nc.allow_low_precision(reason="fp8 attention is intentional"),
            tc.tile_pool(name="persist", bufs=1) as pe_,
            tc.tile_pool(name="pt", bufs=2) as ptp,
            tc.tile_pool(name="tmp", bufs=4) as tmp,
            tc.tile_pool(name="st", bufs=2, space="PSUM") as stp,
            tc.tile_pool(name="acc", bufs=3, space="PSUM") as accp,
            tc.tile_pool(name="zp", bufs=1, space="PSUM") as zpp,
        ):
            # ---------- DMA in ----------
            xf8 = pe_.tile([P, CT, N], dt.float8e4, tag="xf8")
            # first 512 columns of both channel tiles land first so the
            # first conv chunk can start as early as possible
            for t in range(CT):
                nc.sync.dma_start(
                    xf8[:, t, 0:512], xf8_d.ap()[:, t * N : t * N + 512]
                )
            for t in range(CT):
                nc.sync.dma_start(
                    xf8[:, t, 512:N], xf8_d.ap()[:, t * N + 512 : (t + 1) * N]
                )
            wsc = {}
            for nm, d in (("k", wk_d), ("q", wq_d), ("v", wv_d)):
                wsc[nm] = pe_.tile([P, CT, C], dt.float8e4, tag=f"w{nm}",
                                   name=f"w{nm}")
                nc.sync.dma_start(wsc[nm].rearrange("p t o -> p (t o)"), d.ap())
            aux_sb = pe_.tile([P, 8], dt.float32, tag="aux")
            nc.sync.dma_start(aux_sb, aux_d.ap())
            bfq = aux_sb[:, 0:2]
            bfk = aux_sb[:, 2:4]
            xb = pe_.tile([P, CT, NQ], dt.bfloat16, tag="xb")
            nc.sync.dma_start(xb.rearrange("p t n -> p (t n)"), xb_d.ap())

            ones_t = pe_.tile([P, 2, 16], dt.float8e4, tag="ones")
            nc.vector.memset(ones_t, 1.0)
            ones = ones_t[:, :, 0:1]
            ebias = pe_.tile([P, 1], dt.float32, tag="ebias")
            nc.vector.memset(ebias, EBIAS)

            k_sb = pe_.tile([P, CT, N], dt.float8e4, tag="k")
            q_sb = pe_.tile([P, CT, NQ], dt.float8e4, tag="q")
            vT = pe_.tile([P, NJP, 2, C], dt.float8e4, tag="vT")

            # ---------- conv emitters (called interleaved with attention) ----
            def conv_qk(nm, dst, bias, ck):
                # one 512-col chunk for both output-channel halves
                for h in range(CT):
                    cp = stp.tile([P, 512], dt.float32, tag="st",
                                  name=f"c{nm}{h}_{ck}")
                    nc.tensor.matmul(
                        cp,
                        wsc[nm][:, :, h * P : (h + 1) * P],
                        xf8[:, :, ck * 512 : (ck + 1) * 512],
                        start=True, stop=True, perf_mode=DR,
                    )
                    nc.vector.tensor_scalar_add(
                        dst[:, h, ck * 512 : (ck + 1) * 512], cp,
                        bias[:, h : h + 1],
                    )

            def conv_v(jt):
                vp = stp.tile([P, C], dt.float32, tag="st", name=f"cv{jt}")
                nc.tensor.matmul(
                    vp,
                    xf8[:, :, jt * P : (jt + 1) * P],
                    wsc["v"],
                    start=True, stop=True, perf_mode=DR,
                )
                nc.vector.tensor_copy(vT[:, jt // 2, jt % 2, :], vp)

            # ---------- deferred softmax finalize / output ----------
            pend = {}

            def emit_z(ic):
                isl, a_ps, pts, z_ps = pend[ic]
                for jp in range(NJP):
                    nc.tensor.matmul(
                        z_ps, ones, pts[:, jp],
                        start=(jp == 0), stop=(jp == NJP - 1), perf_mode=DR,
                    )
                zc = tmp.tile([1, 3, IC_W], dt.float32, tag="zc", name=f"zc{ic}")
                nc.vector.tensor_copy(zc[:, 0, :], z_ps)
                nc.vector.reciprocal_approx_accurate(
                    zc[:, 1, :], zc[:, 0, :], zc[:, 2, :]
                )
                zb = tmp.tile([P, IC_W], dt.float32, tag="zb", name=f"zb{ic}")
                nc.gpsimd.partition_broadcast(zb, zc[:, 1, :])
                pend[ic] = (isl, a_ps, pts, zb)

            def emit_out(ic):
                isl, a_ps, pts, zb = pend.pop(ic)
                for ch in range(CT):
                    o = tmp.tile([P, IC_W], dt.float32, tag="o",
                                 name=f"o{ic}_{ch}")
                    nc.vector.tensor_mul(o, a_ps[ch], zb)
                    nc.vector.tensor_add(o, o, xb[:, ch, isl])
                    nc.sync.dma_start(out_ap[:, ch, isl], o)

            # ---------- prime: convs needed before ic0 can start ----------
            conv_qk("k", k_sb, bfk, 0)
            conv_qk("q", q_sb, bfq, 0)
            conv_v(0)
            conv_v(1)

            # ---------- attention ----------
            for ic in range(NIC):
                isl = slice(ic * IC_W, (ic + 1) * IC_W)
                a_ps = [
                    accp.tile([P, IC_W], dt.float32, tag="acc",
                              name=f"a{ic}_{c}")
                    for c in range(CT)
                ]
                pts = ptp.tile([P, NJP, 2, IC_W], dt.float8e4, tag="pt",
                               name=f"pt{ic}")
                z_ps = zpp.tile([1, IC_W], dt.float32, tag="z")
                pend[ic] = (isl, a_ps, pts, z_ps)
                for jp in range(NJP):
                    if ic == 0:
                        # drip-feed the remaining conv work between blocks
                        if jp % 2 == 1 and (jp + 1) // 2 < 8:
                            conv_qk("k", k_sb, bfk, (jp + 1) // 2)
                        if 2 <= jp <= 4:
                            conv_qk("q", q_sb, bfq, jp - 1)
                        for jt in (2 * jp + 2, 2 * jp + 3):
                            if jt < 32:
                                conv_v(jt)
                    st = stp.tile([P, 2, IC_W], dt.float32, tag="st")
                    for u in range(2):
                        jt = 2 * jp + u
                        nc.tensor.matmul(
                            st[:, u, :],
                            k_sb[:, :, jt * P : (jt + 1) * P],
                            q_sb[:, :, isl],
                            start=True, stop=True, perf_mode=DR,
                        )
                    nc.scalar.activation(
                        pts[:, jp], st, AF.Exp, bias=ebias, scale=SCALE
                    )
                    for ch in range(CT):
                        nc.tensor.matmul(
                            a_ps[ch],
                            vT[:, jp, :, ch * P : (ch + 1) * P],
                            pts[:, jp],
                            start=(jp == 0), stop=(jp == NJP - 1),
                            perf_mode=DR,
                        )
                    if jp == 1 and ic > 0:
                        emit_z(ic - 1)
                    if jp == 3 and ic > 0:
                        emit_out(ic - 1)
            emit_z(NIC - 1)
            emit_out(NIC - 1)

    nc.compile()
    return nc


def _get_nc():
    if "nc" not in _CACHED:
        _CACHED["nc"] = _build()
    return _CACHED["nc"]


def kernel(x, gn_scale, gn_bias, wq, bq, wk, bk, wv, bv, wp, bp,
           _trace=False, _trace_cores=None):
    try:
        import jax
        if jax.config.jax_compilation_cache_dir is None:
            jax.config.update("jax_compilation_cache_dir",
                              "/tmp/attnblock_jax_cache")
            jax.config.update("jax_persistent_cache_min_compile_time_secs", 1.0)
    except Exception:
        pass
    from concourse.bass_utils import run_bass_kernel_spmd

    nc = _get_nc()
    x = np.asarray(x, np.float64).reshape(B, C, N)
    gn_scale = np.asarray(gn_scale, np.float64)
    gn_bias = np.asarray(gn_bias, np.float64)
    wq64 = np.asarray(wq, np.float64)
    wk64 = np.asarray(wk, np.float64)
    wv64 = np.asarray(wv, np.float64)
    wp64 = np.asarray(wp, np.float64)
    bq64 = np.asarray(bq, np.float64)
    bk64 = np.asarray(bk, np.float64)
    bv64 = np.asarray(bv, np.float64)
    bp64 = np.asarray(bp, np.float64)

    # GroupNorm statistics per batch -> per-channel affine (host prep)
    g = NUM_GROUPS
    xg = x.reshape(B, g, C // g, N)
    mean = xg.mean(axis=(2, 3))                    # [B, g]
    var = xg.var(axis=(2, 3))
    rstd = 1.0 / np.sqrt(var + EPS)
    mean_c = np.repeat(mean, C // g, axis=1)       # [B, C]
    rstd_c = np.repeat(rstd, C // g, axis=1)
    alpha = rstd_c * gn_scale[None, :]             # [B, C]
    beta = gn_bias[None, :] - mean_c * alpha       # [B, C]

    wpv = wv64.T @ wp64.T                          # [c_in, o]
    bpbv = bp64 + wp64 @ bv64

    def pack8(a):  # [c, cols] f64 -> [128, 2*cols] fp8 (c = t*128 + p)
        a32 = np.clip(a, -240.0, 240.0).astype(np.float32)
        return np.ascontiguousarray(
            np.concatenate([a32[:P], a32[P:]], axis=1)
        ).astype(F8)

    in_maps = []
    for core in range(8):
        b, qh = core // 2, core % 2
        xl = x[b]
        if qh == 1:
            xl = np.concatenate([xl[:, NQ:], xl[:, :NQ]], axis=1)
        # weights scaled by this batch's GN affine
        wscq = wq64.T * alpha[b][:, None]          # [c_in, o]
        wsck = wk64.T * alpha[b][:, None]
        wscv = wpv * alpha[b][:, None]
        bfq = bq64 + wq64 @ beta[b]
        bfk = bk64 + wk64 @ beta[b]
        bpp = bpbv + wpv.T @ beta[b]
        aux = np.zeros((P, 8), np.float32)
        aux[:, 0] = bfq[:P]; aux[:, 1] = bfq[P:]
        aux[:, 2] = bfk[:P]; aux[:, 3] = bfk[P:]
        xbl = (xl[:, :NQ] + bpp[:, None]).astype(np.float32)
        in_maps.append({
            "xf8": pack8(xl),
            "wscq": pack8(wscq), "wsck": pack8(wsck), "wscv": pack8(wscv),
            "aux": aux,
            "xb": np.ascontiguousarray(
                np.concatenate([xbl[:P], xbl[P:]], axis=1)
            ).astype(BF16),
        })

    last_err = None
    for attempt in range(3):
        try:
            res = run_bass_kernel_spmd(
                nc, in_maps, core_ids=list(range(8)), trace=_trace,
                trace_cores=_trace_cores,
            )
            break
        except Exception as e:  # transient NRT device faults happen rarely
            last_err = e
            import time as _time
            _time.sleep(2.0 * (attempt + 1))
    else:
        raise last_err
    out = np.empty((B, C, N), np.float32)
    for core in range(8):
        b, qh = core // 2, core % 2
        out[b][:, qh * NQ : (qh + 1) * NQ] = res.results[core]["out"]
    if _trace:
        _CACHED["last_results"] = res
    return out.reshape(B, C, H, W)


# revision 7
# speedup vs baseline: 1.0880x; 1.0880x over previous
"""AttnBlock (GroupNorm + single-head spatial self-attention + residual) on
8 Trainium2 NeuronCores.

Sharding: batch (4) x query-half (2) -> 8 independent shards, one per core.
Every core runs the SAME program on different data: the host rolls the
flattened spatial axis by 2048 for odd cores so each core's queries are the
first 2048 columns of its local x, while K/V see the full 4096.

The host precomputes all per-channel affine constants (GroupNorm mean/rstd
folded with gn affine into the conv weights, conv biases folded with the GN
shift, proj+v-bias folded into the residual) so the device does only the
dense work: Q/K/V 1x1 convs, scores, softmax, PV, and the residual add.

Device pipeline (per core), everything fp8e4 DoubleRow on the PE at
256-wide contraction (0.5 cycles/row):
  1. Q/K convs: DR matmuls vs pre-scaled fp8 weights; PSUM->SBUF cast with
     the folded bias on DVE emits q/k directly in fp8 [c_lo, c_hi, n].
  2. V conv emitted transposed [n, c] (lhsT = x tile) -> vT fp8, which is
     also pre-multiplied by the proj weight (wpv = wv^T wp^T), so the PV
     matmul directly produces the proj output.
  3. Attention per 512-query chunk: ST[j,i] = k^T q (DR), one ACT exp per
     256-key pair reads [128,1024] of PSUM and writes fp8 P directly
     (exp(s/16 - 3); the -3 keeps max P ~ 96 << 240 = fp8e4 max, softmax
     shift-invariance cancels it), PV accumulates [c,i] over key pairs.
     Softmax denominator Z via a tiny all-ones [128,2,1] DR stationary
     (out row [1,512]), reciprocal on that row + GpSimd partition
     broadcast, deferred one chunk so the cross-engine latency hides
     under the next chunk's matmul/exp stream.
  4. o = a * (1/Z) + (x + bias) (DVE), DMA out per chunk.
"""
import numpy as np
import ml_dtypes

B, C, H, W = 4, 256, 64, 64
N = H * W            # 4096 spatial positions
NQ = N // 2          # 2048 queries per core
P = 128              # partitions
CT = C // P          # 2 channel tiles
NUM_GROUPS = 8
EPS = 1e-5
SCALE = float(C) ** -0.5
EBIAS = -3.0         # exp(s/16 - 3): max scaled score ~7.6 -> max P ~ 96
IC_W = 512
NIC = NQ // IC_W     # 4 query chunks
NJP = N // 256       # 16 key pairs

F8 = ml_dtypes.float8_e4m3
BF16 = ml_dtypes.bfloat16

_CACHED = {}


def _build():
    import concourse.bass as bass
    import concourse.mybir as mybir
    import concourse.tile as tile
    from concourse import bacc

    dt = mybir.dt
    AF = mybir.ActivationFunctionType
    DR = mybir.MatmulPerfMode.DoubleRow

    nc = bacc.Bacc("TRN2", debug=False, num_devices=8)

    xf8_d = nc.dram_tensor("xf8", [P, CT * N], dt.float8e4, kind="ExternalInput")
    wq_d = nc.dram_tensor("wscq", [P, CT * C], dt.float8e4, kind="ExternalInput")
    wk_d = nc.dram_tensor("wsck", [P, CT * C], dt.float8e4, kind="ExternalInput")
    wv_d = nc.dram_tensor("wscv", [P, CT * C], dt.float8e4, kind="ExternalInput")
    aux_d = nc.dram_tensor("aux", [P, 8], dt.float32, kind="ExternalInput")
    xb_d = nc.dram_tensor("xb", [P, CT * NQ], dt.bfloat16, kind="ExternalInput")
    out_d = nc.dram_tensor("out", [C, NQ], dt.float32, kind="ExternalOutput")

    out_ap = out_d.ap().rearrange("(t p) n -> p t n", p=P)

    with tile.TileContext(nc) as tc:
        with (
            nc.allow_low_precision(reason="fp8 attention is intentional"),
            tc.tile_pool(name="persist", bufs=1) as pe_,
            tc.tile_pool(name="pt", bufs=2) as ptp,
            tc.tile_pool(name="tmp", bufs=4) as tmp,
            tc.tile_pool(name="st", bufs=2, space="PSUM") as stp,
            tc.tile_pool(name="acc", bufs=3, space="PSUM") as accp,
            tc.tile_pool(name="zp", bufs=1, space="PSUM") as zpp,
        ):
            # ---------- DMA in ----------
            # order: first conv chunk of x, then weights/aux (small), then
            # the x bulk, then the residual (needed only at ic0's epilogue)
            xf8 = pe_.tile([P, CT, N], dt.float8e4, tag="xf8")
            for t in range(CT):
                nc.sync.dma_start(
                    xf8[:, t, 0:512], xf8_d.ap()[:, t * N : t * N + 512]
                )
            wsc = {}
            for nm, d in (("k", wk_d), ("q", wq_d), ("v", wv_d)):
                wsc[nm] = pe_.tile([P, CT, C], dt.float8e4, tag=f"w{nm}",
                                   name=f"w{nm}")
                nc.sync.dma_start(wsc[nm].rearrange("p t o -> p (t o)"), d.ap())
            aux_sb = pe_.tile([P, 8], dt.float32, tag="aux")
            nc.sync.dma_start(aux_sb, aux_d.ap())
            for t in range(CT):
                nc.sync.dma_start(
                    xf8[:, t, 512:N], xf8_d.ap()[:, t * N + 512 : (t + 1) * N]
                )
            bfq = aux_sb[:, 0:2]
            bfk = aux_sb[:, 2:4]
            xb = pe_.tile([P, CT, NQ], dt.bfloat16, tag="xb")
            nc.sync.dma_start(xb.rearrange("p t n -> p (t n)"), xb_d.ap())

            ones_t = pe_.tile([P, 2, 16], dt.float8e4, tag="ones")
            nc.vector.memset(ones_t, 1.0)
            ones = ones_t[:, :, 0:1]
            ebias = pe_.tile([P, 1], dt.float32, tag="ebias")
            nc.vector.memset(ebias, EBIAS)

            k_sb = pe_.tile([P, CT, N], dt.float8e4, tag="k")
            q_sb = pe_.tile([P, CT, NQ], dt.float8e4, tag="q")
            vT = pe_.tile([P, NJP, 2, C], dt.float8e4, tag="vT")

            # ---------- conv emitters (called interleaved with attention) ----
            def conv_qk(nm, dst, bias, ck):
                # one 512-col chunk for both output-channel halves
                for h in range(CT):
                    cp = stp.tile([P, 512], dt.float32, tag="st",
                                  name=f"c{nm}{h}_{ck}")
                    nc.tensor.matmul(
                        cp,
                        wsc[nm][:, :, h * P : (h + 1) * P],
                        xf8[:, :, ck * 512 : (ck + 1) * 512],
                        start=True, stop=True, perf_mode=DR,
                    )
                    nc.vector.tensor_scalar_add(
                        dst[:, h, ck * 512 : (ck + 1) * 512], cp,
                        bias[:, h : h + 1],
                    )

            def conv_v(jp):
                # both key tiles of pair jp into one psum bank, one cast
                vp = stp.tile([P, 2, C], dt.float32, tag="st", name=f"cv{jp}")
                for u in range(2):
                    jt = 2 * jp + u
                    nc.tensor.matmul(
                        vp[:, u, :],
                        xf8[:, :, jt * P : (jt + 1) * P],
                        wsc["v"],
                        start=True, stop=True, perf_mode=DR,
                    )
                nc.vector.tensor_copy(vT[:, jp], vp)

            # ---------- attention ----------
            # PV/Z for chunk ic run one chunk late, interleaved into chunk
            # ic+1's ST/exp stream: chunk 0's PE slack absorbs the convs and
            # the softmax finalize latency always hides under live matmuls.
            pend = {}

            def emit_pv(ic, jp):
                a_ps, pts = pend[ic]["a"], pend[ic]["pts"]
                for ch in range(CT):
                    nc.tensor.matmul(
                        a_ps[ch],
                        vT[:, jp, :, ch * P : (ch + 1) * P],
                        pts[:, jp],
                        start=(jp == 0), stop=(jp == NJP - 1),
                        perf_mode=DR,
                    )

            def emit_z2(ic, zjp):
                pts, z_ps = pend[ic]["pts"], pend[ic]["z"]
                for jp in (zjp, zjp + 1):
                    nc.tensor.matmul(
                        z_ps, ones, pts[:, jp],
                        start=(jp == 0), stop=(jp == NJP - 1), perf_mode=DR,
                    )

            def emit_fin(ic):
                z_ps = pend[ic]["z"]
                zc = tmp.tile([1, 3, IC_W], dt.float32, tag="zc", name=f"zc{ic}")
                nc.vector.tensor_copy(zc[:, 0, :], z_ps)
                nc.vector.reciprocal_approx_accurate(
                    zc[:, 1, :], zc[:, 0, :], zc[:, 2, :]
                )
                zb = tmp.tile([P, IC_W], dt.float32, tag="zb", name=f"zb{ic}")
                nc.gpsimd.partition_broadcast(zb, zc[:, 1, :])
                pend[ic]["zb"] = zb

            def emit_out(ic):
                st_ = pend.pop(ic)
                isl, a_ps, zb = st_["isl"], st_["a"], st_["zb"]
                for ch in range(CT):
                    o = tmp.tile([P, IC_W], dt.float32, tag="o",
                                 name=f"o{ic}_{ch}")
                    nc.vector.tensor_mul(o, a_ps[ch], zb)
                    nc.vector.tensor_add(o, o, xb[:, ch, isl])
                    nc.sync.dma_start(out_ap[:, ch, isl], o)

            def deferred(ic, jp):
                # PV/Z/finalize work for chunk ic, paced by chunk ic+1's jps
                if ic < 0:
                    return
                if jp == 0:
                    pend[ic]["a"] = [
                        accp.tile([P, IC_W], dt.float32, tag="acc",
                                  name=f"a{ic}_{c}")
                        for c in range(CT)
                    ]
                    pend[ic]["z"] = zpp.tile([1, IC_W], dt.float32, tag="z",
                                             name=f"z{ic}")
                emit_pv(ic, jp)
                if 1 <= jp <= 8:
                    emit_z2(ic, 2 * (jp - 1))
                if jp == 9:
                    emit_fin(ic)
                if jp == NJP - 1:
                    emit_out(ic)

            # prime: convs needed before chunk 0 can start
            conv_qk("k", k_sb, bfk, 0)
            conv_qk("q", q_sb, bfq, 0)
            conv_v(0)

            for ic in range(NIC):
                isl = slice(ic * IC_W, (ic + 1) * IC_W)
                pts = ptp.tile([P, NJP, 2, IC_W], dt.float8e4, tag="pt",
                               name=f"pt{ic}")
                pend[ic] = {"isl": isl, "pts": pts}
                for jp in range(NJP):
                    if ic == 0:
                        # drip-feed the remaining conv work between blocks
                        if jp % 2 == 1 and (jp + 1) // 2 < 8:
                            conv_qk("k", k_sb, bfk, (jp + 1) // 2)
                        if 2 <= jp <= 4:
                            conv_qk("q", q_sb, bfq, jp - 1)
                        if jp + 1 < NJP:
                            conv_v(jp + 1)
                    st = stp.tile([P, 2, IC_W], dt.float32, tag="st")
                    for u in range(2):
                        jt = 2 * jp + u
                        nc.tensor.matmul(
                            st[:, u, :],
                            k_sb[:, :, jt * P : (jt + 1) * P],
                            q_sb[:, :, isl],
                            start=True, stop=True, perf_mode=DR,
                        )
                    nc.scalar.activation(
                        pts[:, jp], st, AF.Exp, bias=ebias, scale=SCALE
                    )
                    deferred(ic - 1, jp)
            # drain: deferred work for the last chunk
            for jp in range(NJP):
                deferred(NIC - 1, jp)

    nc.compile()
    return nc


def _get_nc():
    if "nc" not in _CACHED:
        _CACHED["nc"] = _build()
    return _CACHED["nc"]


def kernel(x, gn_scale, gn_bias, wq, bq, wk, bk, wv, bv, wp, bp,
           _trace=False, _trace_cores=None):
    try:
        import jax
        if jax.config.jax_compilation_cache_dir is None:
            jax.config.update("jax_compilation_cache_dir",
                              "/tmp/attnblock_jax_cache")
            jax.config.update("jax_persistent_cache_min_compile_time_secs", 1.0)
    except Exception:
        pass
    from concourse.bass_utils import run_bass_kernel_spmd

    nc = _get_nc()
    x = np.asarray(x, np.float64).reshape(B, C, N)
    gn_scale = np.asarray(gn_scale, np.float64)
    gn_bias = np.asarray(gn_bias, np.float64)
    wq64 = np.asarray(wq, np.float64)
    wk64 = np.asarray(wk, np.float64)
    wv64 = np.asarray(wv, np.float64)
    wp64 = np.asarray(wp, np.float64)
    bq64 = np.asarray(bq, np.float64)
    bk64 = np.asarray(bk, np.float64)
    bv64 = np.asarray(bv, np.float64)
    bp64 = np.asarray(bp, np.float64)

    # GroupNorm statistics per batch -> per-channel affine (host prep)
    g = NUM_GROUPS
    xg = x.reshape(B, g, C // g, N)
    mean = xg.mean(axis=(2, 3))                    # [B, g]
    var = xg.var(axis=(2, 3))
    rstd = 1.0 / np.sqrt(var + EPS)
    mean_c = np.repeat(mean, C // g, axis=1)       # [B, C]
    rstd_c = np.repeat(rstd, C // g, axis=1)
    alpha = rstd_c * gn_scale[None, :]             # [B, C]
    beta = gn_bias[None, :] - mean_c * alpha       # [B, C]

    wpv = wv64.T @ wp64.T                          # [c_in, o]
    bpbv = bp64 + wp64 @ bv64

    def pack8(a):  # [c, cols] f64 -> [128, 2*cols] fp8 (c = t*128 + p)
        a32 = np.clip(a, -240.0, 240.0).astype(np.float32)
        return np.ascontiguousarray(
            np.concatenate([a32[:P], a32[P:]], axis=1)
        ).astype(F8)

    in_maps = []
    for core in range(8):
        b, qh = core // 2, core % 2
        xl = x[b]
        if qh == 1:
            xl = np.concatenate([xl[:, NQ:], xl[:, :NQ]], axis=1)
        # weights scaled by this batch's GN affine
        wscq = wq64.T * alpha[b][:, None]          # [c_in, o]
        wsck = wk64.T * alpha[b][:, None]
        wscv = wpv * alpha[b][:, None]
        bfq = bq64 + wq64 @ beta[b]
        bfk = bk64 + wk64 @ beta[b]
        bpp = bpbv + wpv.T @ beta[b]
        aux = np.zeros((P, 8), np.float32)
        aux[:, 0] = bfq[:P]; aux[:, 1] = bfq[P:]
        aux[:, 2] = bfk[:P]; aux[:, 3] = bfk[P:]
        xbl = (xl[:, :NQ] + bpp[:, None]).astype(np.float32)
        in_maps.append({
            "xf8": pack8(xl),
            "wscq": pack8(wscq), "wsck": pack8(wsck), "wscv": pack8(wscv),
            "aux": aux,
            "xb": np.ascontiguousarray(
                np.concatenate([xbl[:P], xbl[P:]], axis=1)
            ).astype(BF16),
        })

    last_err = None
    for attempt in range(3):
        try:
            res = run_bass_kernel_spmd(
                nc, in_maps, core_ids=list(range(8)), trace=_trace,
                trace_cores=_trace_cores,
            )
            break
        except Exception as e:  # transient NRT device faults happen rarely
            last_err = e
            import time as _time
            _time.sleep(2.0 * (attempt + 1))
    else:
        raise last_err
    out = np.empty((B, C, N), np.float32)
    for core in range(8):
        b, qh = core // 2, core % 2
        out[b][:, qh * NQ : (qh + 1) * NQ] = res.results[core]["out"]
    if _trace:
        _CACHED["last_results"] = res
    return out.reshape(B, C, H, W)
